# revision 1
# baseline (speedup 1.0000x reference)
"""NT-Xent (SimCLR contrastive) loss on Trainium2, sharded across 8 NeuronCores.

Algorithm (per core c of 8, SPMD — same program, per-core input data):
  - Host ships z^T = concat(z_i, z_j).T as bf16 [512, 4096] (shared), plus the
    core's own 512-row column slice and its positive-partner slice.
  - Device computes column norms via squares + all-ones matmul (partition
    reduction, result broadcast across partitions for free), rinv =
    exp(-0.5*ln(ssq)), normalizes to zn^T, then computes the core's
    [512, 4096] slice of the cosine-similarity Gram matrix with bf16 matmuls.
  - exp(10*sim) row-sums are accumulated by ScalarE's fused activation
    accumulator; the masked diagonal is handled by subtracting exp(10)
    (diag of the normalized Gram is 1.0 by construction).
  - Positive-pair similarities come from an elementwise product + ones-matmul.
  - Each core emits one scalar partial: sum_r ln(Z_r) - 10 * sum_r pos_r.
  - Host: loss = sum(partials) / 4096.
"""

import numpy as np

B = 2048
D = 512
N2 = 2 * B              # 4096 total rows
NCORES = 8
RPC = N2 // NCORES      # 512 rows per core
KT = D // 128           # 4 contraction tiles
NB = N2 // 512          # 8 column blocks of 512
TEMP = 0.1
SCALE = 1.0 / TEMP      # 10.0
EXP_DIAG = float(np.exp(np.float64(SCALE)))  # exp(10), the masked diagonal term

_CACHE = {}


def _build():
    from concourse import bass, bacc, tile, mybir

    nc = bacc.Bacc("TRN2", target_bir_lowering=False, debug=False,
                   num_devices=NCORES)
    bf16 = mybir.dt.bfloat16
    f32 = mybir.dt.float32
    F = mybir.ActivationFunctionType
    A = mybir.AluOpType
    AX = mybir.AxisListType
    PSUM = bass.MemorySpace.PSUM

    zt = nc.dram_tensor("zt", [D, N2], bf16, kind="ExternalInput").ap()
    zown = nc.dram_tensor("zown", [D, RPC], bf16, kind="ExternalInput").ap()
    zpr = nc.dram_tensor("zpr", [D, RPC], bf16, kind="ExternalInput").ap()
    out = nc.dram_tensor("out", [1, 1], f32, kind="ExternalOutput").ap()

    with tile.TileContext(nc) as tc:
        with (
            tc.tile_pool(name="sb", bufs=1) as sb,
            tc.tile_pool(name="wrk", bufs=2) as wrk,
        ):
            ones = sb.tile([128, 128], bf16, tag="ones")
            nc.vector.memset(ones[:], 1.0)

            # ---- load z^T (full, shared) and the own/partner slices ----
            ztk = []
            for k in range(KT):
                t = sb.tile([128, N2], bf16, tag=f"zt{k}")
                nc.sync.dma_start(out=t[:], in_=zt[k * 128:(k + 1) * 128, :])
                ztk.append(t)

            def load_slices(src):
                ts = []
                for k in range(KT):
                    t = sb.tile([128, RPC], bf16, tag=f"{src.name}{k}")
                    nc.sync.dma_start(out=t[:], in_=src[k * 128:(k + 1) * 128, :])
                    ts.append(t)
                return ts

            zok = load_slices(zown)
            zpk = load_slices(zpr)

            # ---- column norms of full z^T ----
            sqk = []
            for k in range(KT):
                s = sb.tile([128, N2], bf16, tag=f"sq{k}")
                nc.vector.tensor_tensor(s[:], ztk[k][:], ztk[k][:], A.mult)
                sqk.append(s)

            lnssq = sb.tile([128, N2], f32, tag="lnssq")
            rinv = sb.tile([128, N2], bf16, tag="rinv")
            with tc.tile_pool(name="psA", bufs=2, space=PSUM) as psA:
                for n in range(NB):
                    sl = slice(n * 512, (n + 1) * 512)
                    ps = psA.tile([128, 512], f32, tag="ssq")
                    for k in range(KT):
                        nc.tensor.matmul(ps[:], ones[:], sqk[k][:, sl],
                                         start=(k == 0), stop=(k == KT - 1))
                    nc.scalar.activation(lnssq[:, sl], ps[:], F.Ln)
                nc.scalar.activation(rinv[:], lnssq[:], F.Exp, scale=-0.5)

                # ---- normalize ----
                zntk = []
                for k in range(KT):
                    zn = sb.tile([128, N2], bf16, tag=f"znt{k}")
                    nc.vector.tensor_tensor(zn[:], ztk[k][:], rinv[:], A.mult)
                    zntk.append(zn)

                # own/partner norms (recomputed from the slices so the SPMD
                # program stays core-independent; bit-identical to slices of
                # the full-z path)
                def norm_small(tks, tag):
                    lns = wrk.tile([128, RPC], f32, tag="lns_s")
                    rin = sb.tile([128, RPC], bf16, tag=f"rin_{tag}")
                    ps = psA.tile([128, 512], f32, tag="ssq_s")
                    for k in range(KT):
                        s = wrk.tile([128, RPC], bf16, tag="sq_s")
                        nc.vector.tensor_tensor(s[:], tks[k][:], tks[k][:], A.mult)
                        nc.tensor.matmul(ps[:], ones[:], s[:],
                                         start=(k == 0), stop=(k == KT - 1))
                    nc.scalar.activation(lns[:], ps[:], F.Ln)
                    nc.scalar.activation(rin[:], lns[:], F.Exp, scale=-0.5)
                    zn = []
                    for k in range(KT):
                        t = sb.tile([128, RPC], bf16, tag=f"zn_{tag}{k}")
                        nc.vector.tensor_tensor(t[:], tks[k][:], rin[:], A.mult)
                        zn.append(t)
                    return zn

                znok = norm_small(zok, "o")
                znpk = norm_small(zpk, "p")

                # ---- positive-pair cosines: pos[r] = sum_d zno[d,r]*znp[d,r]
                pp = psA.tile([128, 512], f32, tag="pos")
                for k in range(KT):
                    pr = wrk.tile([128, RPC], bf16, tag="prod")
                    nc.vector.tensor_tensor(pr[:], znok[k][:], znpk[k][:], A.mult)
                    nc.tensor.matmul(pp[:], ones[:], pr[:],
                                     start=(k == 0), stop=(k == KT - 1))
                pos_red = sb.tile([128, 1], f32, tag="posr")
                nc.vector.tensor_reduce(pos_red[:], pp[:], AX.X, A.add)

            # ---- main Gram slice + fused exp row-sums ----
            rowp = sb.tile([128, 8], f32, tag="rowp")
            with tc.tile_pool(name="psB", bufs=2, space=PSUM) as psB:
                for m in range(4):
                    for h in range(2):
                        pm = psB.tile([128, 2048], f32, tag="mm")
                        for k in range(KT):
                            lhsT = znok[k][:, m * 128:(m + 1) * 128]
                            for n4 in range(4):
                                col = h * 2048 + n4 * 512
                                nc.tensor.matmul(
                                    pm[:, n4 * 512:(n4 + 1) * 512],
                                    lhsT, zntk[k][:, col:col + 512],
                                    start=(k == 0), stop=(k == KT - 1))
                        scr = wrk.tile([128, 2048], f32, tag="scr")
                        j = m * 2 + h
                        nc.scalar.activation(scr[:], pm[:], F.Exp, scale=SCALE,
                                             accum_out=rowp[:, j:j + 1])

            # ---- finale: partial = sum_r ln(Z_r) - 10 * sum_r pos_r ----
            zsum = sb.tile([128, 8], f32, tag="zsum")
            logz = sb.tile([128, 8], f32, tag="logz")
            nc.vector.memset(logz[:], 0.0)
            for m in range(4):
                nc.vector.scalar_tensor_tensor(
                    zsum[:, m:m + 1], rowp[:, 2 * m:2 * m + 1], -EXP_DIAG,
                    rowp[:, 2 * m + 1:2 * m + 2], A.add, A.add)
            nc.scalar.activation(logz[:, 0:4], zsum[:, 0:4], F.Ln)
            nc.vector.tensor_scalar_mul(logz[:, 4:5], pos_red[:], -SCALE / 128.0)
            red1 = sb.tile([128, 1], f32, tag="red1")
            nc.vector.tensor_reduce(red1[:], logz[:], AX.X, A.add)
            fin = sb.tile([1, 1], f32, tag="fin")
            nc.gpsimd.tensor_reduce(fin[:], red1[:], AX.C, A.add)
            nc.sync.dma_start(out=out, in_=fin[:])

    nc.compile()
    return nc


def _get_nc():
    if "nc" not in _CACHE:
        _CACHE["nc"] = _build()
    return _CACHE["nc"]


def _in_maps(z_i, z_j):
    import ml_dtypes

    z = np.concatenate(
        [np.asarray(z_i, np.float32), np.asarray(z_j, np.float32)], axis=0)
    zt = np.ascontiguousarray(z.T).astype(ml_dtypes.bfloat16)
    maps = []
    for c in range(NCORES):
        o = c * RPC
        po = (o + B) % N2
        maps.append({
            "zt": zt,
            "zown": np.ascontiguousarray(zt[:, o:o + RPC]),
            "zpr": np.ascontiguousarray(zt[:, po:po + RPC]),
        })
    return maps


def _run(z_i, z_j, trace=False):
    from concourse.bass_utils import run_bass_kernel_spmd

    nc = _get_nc()
    return run_bass_kernel_spmd(nc, _in_maps(z_i, z_j), list(range(NCORES)),
                                trace=trace)


def kernel(z_i, z_j):
    res = _run(z_i, z_j, trace=False)
    total = sum(float(r["out"][0, 0]) for r in res.results)
    return np.float32(total / N2)


# revision 3
# speedup vs baseline: 1.1166x; 1.1166x over previous
"""NT-Xent (SimCLR contrastive) loss on Trainium2, sharded across 8 NeuronCores.

Algorithm (per core c of 8, SPMD — same program, per-core input data):
  - Host ships z^T = concat(z_i, z_j).T as bf16 [512, 4096] (shared), plus the
    core's own 512-row column slice and its positive-partner slice.
  - Device computes column norms via squares + all-ones matmul (partition
    reduction, result broadcast across partitions for free), rinv =
    exp(-0.5*ln(ssq)), normalizes to zn^T, then computes the core's
    [512, 4096] slice of the cosine-similarity Gram matrix with bf16 matmuls.
  - exp(10*sim) row-sums are accumulated by ScalarE's fused activation
    accumulator; the masked diagonal is handled by subtracting exp(10)
    (diag of the normalized Gram is 1.0 by construction).
  - Positive-pair similarities come from an elementwise product + ones-matmul.
  - Each core emits one scalar partial: sum_r ln(Z_r) - 10 * sum_r pos_r.
  - Host: loss = sum(partials) / 4096.
"""

import numpy as np

B = 2048
D = 512
N2 = 2 * B              # 4096 total rows
NCORES = 8
RPC = N2 // NCORES      # 512 rows per core
KT = D // 128           # 4 contraction tiles
BLK = 1024              # column-block size for the norm pipeline
NBLK = N2 // BLK        # 4 blocks
TEMP = 0.1
SCALE = 1.0 / TEMP      # 10.0
EXP_DIAG = float(np.exp(np.float64(SCALE)))  # exp(10), the masked diagonal term

_CACHE = {}


def _patch_act_tables(nc, mybir):
    """Make Ln and Exp resolve to the shared natural_log_exp_and_others set
    so the compiler emits one ACT table load instead of thrashing between
    exp_and_others and natural_log (~1.3us per reload)."""
    from concourse import hw_specs

    tables = hw_specs.get_activation_tables(nc.m.arch)
    keep = "natural_log_exp_and_others"
    if keep not in tables:
        return
    F = mybir.ActivationFunctionType
    if F.Exp not in tables[keep] or F.Ln not in tables[keep]:
        return
    for name, fns in tables.items():
        if name != keep:
            fns.discard(F.Exp)
            fns.discard(F.Ln)


def _build():
    from concourse import bass, bacc, tile, mybir

    nc = bacc.Bacc("TRN2", target_bir_lowering=False, debug=False,
                   num_devices=NCORES)
    bf16 = mybir.dt.bfloat16
    f32 = mybir.dt.float32
    F = mybir.ActivationFunctionType
    A = mybir.AluOpType
    AX = mybir.AxisListType
    PSUM = bass.MemorySpace.PSUM

    zt = nc.dram_tensor("zt", [D, N2], bf16, kind="ExternalInput").ap()
    zown = nc.dram_tensor("zown", [D, RPC], bf16, kind="ExternalInput").ap()
    zpr = nc.dram_tensor("zpr", [D, RPC], bf16, kind="ExternalInput").ap()
    out = nc.dram_tensor("out", [1, 1], f32, kind="ExternalOutput").ap()

    with tile.TileContext(nc) as tc:
        with (
            tc.tile_pool(name="sb", bufs=1) as sb,
            tc.tile_pool(name="wrk", bufs=2) as wrk,
        ):
            ones = sb.tile([128, 128], bf16, tag="ones")
            nc.vector.memset(ones[:], 1.0)

            # ---- own/partner slices first: the main matmul's stationary
            # operand depends on these, so get them normalized ASAP ----
            def load_slices(src, tag):
                ts = []
                for k in range(KT):
                    t = sb.tile([128, RPC], bf16, tag=f"{tag}{k}")
                    nc.sync.dma_start(out=t[:], in_=src[k * 128:(k + 1) * 128, :])
                    ts.append(t)
                return ts

            zok = load_slices(zown, "zo")
            zpk = load_slices(zpr, "zp")

            with tc.tile_pool(name="psA", bufs=2, space=PSUM) as psA:

                def norm_small(tks, tag):
                    lns = wrk.tile([128, RPC], f32, tag="lns_s")
                    rin = sb.tile([128, RPC], bf16, tag=f"rin_{tag}")
                    ps = psA.tile([128, 512], f32, tag="ssq_s")
                    for k in range(KT):
                        s = wrk.tile([128, RPC], bf16, tag="sq_s")
                        nc.vector.tensor_tensor(s[:], tks[k][:], tks[k][:], A.mult)
                        nc.tensor.matmul(ps[:], ones[:], s[:],
                                         start=(k == 0), stop=(k == KT - 1))
                    nc.scalar.activation(lns[:], ps[:], F.Ln)
                    nc.scalar.activation(rin[:], lns[:], F.Exp, scale=-0.5)
                    zn = []
                    for k in range(KT):
                        t = sb.tile([128, RPC], bf16, tag=f"zn_{tag}{k}")
                        nc.vector.tensor_tensor(t[:], tks[k][:], rin[:], A.mult)
                        zn.append(t)
                    return zn

                znok = norm_small(zok, "o")
                znpk = norm_small(zpk, "p")

                # positive-pair cosines: pos[r] = sum_d zno[d,r]*znp[d,r]
                pp = psA.tile([128, 512], f32, tag="pos")
                for k in range(KT):
                    pr = wrk.tile([128, RPC], bf16, tag="prod")
                    nc.vector.tensor_tensor(pr[:], znok[k][:], znpk[k][:], A.mult)
                    nc.tensor.matmul(pp[:], ones[:], pr[:],
                                     start=(k == 0), stop=(k == KT - 1))
                pos_red = sb.tile([128, 1], f32, tag="posr")
                nc.vector.tensor_reduce(pos_red[:], pp[:], AX.X, A.add)

                # ---- full z^T: per-block pipeline (load, square, colsum,
                # rinv, normalize) so main matmuls can start on block 0 ----
                zblk = [[None] * KT for _ in range(NBLK)]   # raw bf16
                znt = [[None] * KT for _ in range(NBLK)]    # normalized bf16
                for b in range(NBLK):
                    bsl = slice(b * BLK, (b + 1) * BLK)
                    for k in range(KT):
                        t = sb.tile([128, BLK], bf16, tag=f"zt{b}_{k}")
                        nc.sync.dma_start(out=t[:], in_=zt[k * 128:(k + 1) * 128, bsl])
                        zblk[b][k] = t
                    ps = psA.tile([128, BLK], f32, tag="ssq")
                    for k in range(KT):
                        s = wrk.tile([128, BLK], bf16, tag="sq")
                        nc.vector.tensor_tensor(s[:], zblk[b][k][:], zblk[b][k][:],
                                                A.mult)
                        for j in range(BLK // 512):
                            nc.tensor.matmul(ps[:, j * 512:(j + 1) * 512],
                                             ones[:], s[:, j * 512:(j + 1) * 512],
                                             start=(k == 0), stop=(k == KT - 1))
                    lns = wrk.tile([128, BLK], f32, tag="lns")
                    nc.scalar.activation(lns[:], ps[:], F.Ln)
                    rin = wrk.tile([128, BLK], bf16, tag="rin")
                    nc.scalar.activation(rin[:], lns[:], F.Exp, scale=-0.5)
                    for k in range(KT):
                        t = sb.tile([128, BLK], bf16, tag=f"znt{b}_{k}")
                        nc.vector.tensor_tensor(t[:], zblk[b][k][:], rin[:], A.mult)
                        znt[b][k] = t

            # ---- main Gram slice + fused exp row-sums ----
            rowp = sb.tile([128, 8], f32, tag="rowp")
            with tc.tile_pool(name="psB", bufs=2, space=PSUM) as psB:
                for h in range(2):          # column half (2048 cols)
                    for m in range(4):      # own-row tile
                        pm = psB.tile([128, 2048], f32, tag="mm")
                        for k in range(KT):
                            lhsT = znok[k][:, m * 128:(m + 1) * 128]
                            for n4 in range(4):
                                col = h * 2048 + n4 * 512
                                nc.tensor.matmul(
                                    pm[:, n4 * 512:(n4 + 1) * 512],
                                    lhsT,
                                    znt[col // BLK][k][:, col % BLK:col % BLK + 512],
                                    start=(k == 0), stop=(k == KT - 1))
                        scr = wrk.tile([128, 2048], f32, tag="scr")
                        j = m * 2 + h
                        nc.scalar.activation(scr[:], pm[:], F.Exp, scale=SCALE,
                                             accum_out=rowp[:, j:j + 1])

            # ---- finale: partial = sum_r ln(Z_r) - 10 * sum_r pos_r ----
            zsum = sb.tile([128, 8], f32, tag="zsum")
            logz = sb.tile([128, 8], f32, tag="logz")
            nc.vector.memset(logz[:], 0.0)
            for m in range(4):
                nc.vector.scalar_tensor_tensor(
                    zsum[:, m:m + 1], rowp[:, 2 * m:2 * m + 1], -EXP_DIAG,
                    rowp[:, 2 * m + 1:2 * m + 2], A.add, A.add)
            nc.scalar.activation(logz[:, 0:4], zsum[:, 0:4], F.Ln)
            nc.vector.tensor_scalar_mul(logz[:, 4:5], pos_red[:], -SCALE / 128.0)
            red1 = sb.tile([128, 1], f32, tag="red1")
            nc.vector.tensor_reduce(red1[:], logz[:], AX.X, A.add)
            fin = sb.tile([1, 1], f32, tag="fin")
            nc.gpsimd.tensor_reduce(fin[:], red1[:], AX.C, A.add)
            nc.sync.dma_start(out=out, in_=fin[:])

    _patch_act_tables(nc, mybir)
    nc.compile()
    return nc


def _get_nc():
    if "nc" not in _CACHE:
        _CACHE["nc"] = _build()
    return _CACHE["nc"]


def _in_maps(z_i, z_j):
    import ml_dtypes

    z = np.concatenate(
        [np.asarray(z_i, np.float32), np.asarray(z_j, np.float32)], axis=0)
    zt = np.ascontiguousarray(z.T).astype(ml_dtypes.bfloat16)
    maps = []
    for c in range(NCORES):
        o = c * RPC
        po = (o + B) % N2
        maps.append({
            "zt": zt,
            "zown": np.ascontiguousarray(zt[:, o:o + RPC]),
            "zpr": np.ascontiguousarray(zt[:, po:po + RPC]),
        })
    return maps


def _run(z_i, z_j, trace=False):
    from concourse.bass_utils import run_bass_kernel_spmd

    nc = _get_nc()
    return run_bass_kernel_spmd(nc, _in_maps(z_i, z_j), list(range(NCORES)),
                                trace=trace)


def kernel(z_i, z_j):
    res = _run(z_i, z_j, trace=False)
    total = sum(float(r["out"][0, 0]) for r in res.results)
    return np.float32(total / N2)
